# revision 12
# baseline (speedup 1.0000x reference)
"""Trainium2 Bass kernel for nn_Compressor (consecutive-run mean-pool compressor).

Semantics per batch element (T=4096, D=1024, blank_idx=0):
  - split preds[b] into consecutive runs
  - out[s] = mean(x[start_s : start_s+count_s]) for run s, zeroed when the
    run's label == 0; rows s >= n_runs stay zero (padding to T rows).

Strategy (pure data parallel, 2 examples per core on 8 cores):
  preds is tiny (256 KB) -> run metadata is computed on host with numpy and
  shipped as small index/weight tensors; the 512 MB of x moves on-device.
  Each core's x shard gets one extra all-zero row appended (index ROWS)
  used as a null target for padding gathers.

  Since preds is uniform over 32 labels, ~97% of runs have length 1, so the
  output is nearly a row-gather of x. Per 512 output rows (one block-tile):
    - ONE indirect DMA gathers 128 blocks of L=4 consecutive x rows (16 KB
      per descriptor, keeping SWDGE descriptor generation cheap). Block
      anchors are majority-voted on host so ~97% of output rows are covered
      by the block slot at their position.
    - 4 per-slot DVE multiplies apply w_s = (label!=0)/count_s (0 for
      blank/padding/mismatched rows).
    - ONE blocked 2 MB store (16 KB per descriptor).
  Output rows not fully covered by their slot (anchor mismatch, or runs
  with count>1) get the missing contribution via a compact path: entries
  are grouped by destination row (<=128 unique dsts per group, sorted by
  descending row count); each group's row-sums are built with a plain
  gather plus accumulate-gathers (DMA CCE add into SBUF; exhausted rows
  point at the zero row), scaled once by w, then indirect scatter-added
  (CCE add into DRAM) after the base stores.
"""

import numpy as np

B, T, D = 16, 4096, 1024
N_CORES = 8
EX_PER_CORE = B // N_CORES  # 2
ROWS = EX_PER_CORE * T  # 8192 rows of out per core
XR = ROWS + 1  # x shard rows incl. trailing zero row
ZROW = ROWS  # index of the zero row
L = 2  # rows per gather/store block
NTB = ROWS // (128 * L)  # 16 block-tiles per core
BLANK_IDX = 0

_BUILD_CACHE: dict = {}


def _build(rounds_per_group: tuple):
    """Build + compile the Bass kernel. rounds_per_group[g] = number of
    gather rounds (1 plain + R-1 accumulate) for extras group g."""
    import concourse.bass as bass
    import concourse.tile as tile
    from concourse import bacc, mybir
    # add_dep_helper no longer needed

    n_groups = len(rounds_per_group)
    n_round_cols = int(sum(rounds_per_group))

    nc = bacc.Bacc(
        "TRN2", target_bir_lowering=False, debug=False, enable_asserts=False
    )
    f32, i32 = mybir.dt.float32, mybir.dt.int32

    x_d = nc.dram_tensor("x", [XR, D], f32, kind="ExternalInput").ap()
    bidx_d = nc.dram_tensor("bidx", [128, NTB], i32, kind="ExternalInput").ap()
    bw_d = nc.dram_tensor("bw", [128, NTB * L], f32, kind="ExternalInput").ap()
    out_d = nc.dram_tensor("out", [ROWS, D], f32, kind="ExternalOutput").ap()
    if n_groups:
        eq_d = nc.dram_tensor(
            "eq", [128, n_round_cols], i32, kind="ExternalInput"
        ).ap()
        ew_d = nc.dram_tensor("ew", [128, n_groups], f32, kind="ExternalInput").ap()
        ed_d = nc.dram_tensor("ed", [128, n_groups], i32, kind="ExternalInput").ap()

    with tile.TileContext(nc) as tc:
        with (
            tc.tile_pool(name="const", bufs=1) as cpool,
            tc.tile_pool(name="gather", bufs=10) as gpool,
            tc.tile_pool(name="extras", bufs=max(1, n_groups)) as epool,
        ):
            idx_sb = cpool.tile([128, NTB], i32)
            nc.sync.dma_start(idx_sb[:], bidx_d[:])
            w_sb = cpool.tile([128, NTB * L], f32)
            nc.sync.dma_start(w_sb[:], bw_d[:])
            if n_groups:
                eq_sb = cpool.tile([128, n_round_cols], i32)
                nc.sync.dma_start(eq_sb[:], eq_d[:])
                ew_sb = cpool.tile([128, n_groups], f32)
                nc.sync.dma_start(ew_sb[:], ew_d[:])
                ed_sb = cpool.tile([128, n_groups], i32)
                nc.sync.dma_start(ed_sb[:], ed_d[:])

            # interleave schedule: emit extras gather rounds early in the
            # main loop so the gpsimd engine issues them while store slots
            # are still draining, not after all base gathers.
            round_list = []  # (group, round_t, col)
            col = 0
            for gi in range(n_groups):
                for t in range(rounds_per_group[gi]):
                    round_list.append((gi, t, col))
                    col += 1
            # spread each group's rounds >=n_groups tiles apart: round t of
            # group gi goes after tile 1+gi+t*n_groups, so the serial RMW
            # chain on a group's tile never stalls the in-order gpsimd
            # issue stream (the previous round has long completed).
            rounds_after_tile = {}
            for gi, t, cc in round_list:
                slot = 1 + gi + t * max(1, n_groups)
                rounds_after_tile.setdefault(slot, []).append((gi, t, cc))

            ge_tiles = [None] * n_groups
            mul_done = set()

            # main loop: blocked gather -> per-slot scale -> blocked store
            for j in range(NTB):
                g = gpool.tile([128, L * D], f32)
                nc.gpsimd.indirect_dma_start(
                    out=g[:],
                    out_offset=None,
                    in_=x_d[:],
                    in_offset=bass.IndirectOffsetOnAxis(
                        ap=idx_sb[:, j : j + 1], axis=0
                    ),
                )
                for l in range(L):
                    nc.vector.tensor_scalar_mul(
                        out=g[:, l * D : (l + 1) * D],
                        in0=g[:, l * D : (l + 1) * D],
                        scalar1=w_sb[:, j * L + l : j * L + l + 1],
                    )
                nc.sync.dma_start(
                    out_d[j * 128 * L : (j + 1) * 128 * L, :].rearrange(
                        "(p l) d -> p (l d)", l=L
                    ),
                    g[:],
                )
                for gi, t, cc in rounds_after_tile.get(j, []):
                    if ge_tiles[gi] is None:
                        ge_tiles[gi] = epool.tile([128, D], f32, name=f"ge{gi}", tag=f"ge{gi}")
                    nc.gpsimd.indirect_dma_start(
                        out=ge_tiles[gi][:],
                        out_offset=None,
                        in_=x_d[:],
                        in_offset=bass.IndirectOffsetOnAxis(
                            ap=eq_sb[:, cc : cc + 1], axis=0
                        ),
                        compute_op=(
                            mybir.AluOpType.bypass if t == 0 else mybir.AluOpType.add
                        ),
                    )
                    if t == rounds_per_group[gi] - 1:
                        nc.vector.tensor_scalar_mul(
                            out=ge_tiles[gi][:],
                            in0=ge_tiles[gi][:],
                            scalar1=ew_sb[:, gi : gi + 1],
                        )
                        mul_done.add(gi)

            # any rounds not emitted during the loop (slot beyond last tile)
            for gi, t, cc in round_list:
                slot = 1 + gi + t * max(1, n_groups)
                if slot <= NTB - 1:
                    continue
                if ge_tiles[gi] is None:
                    ge_tiles[gi] = epool.tile([128, D], f32, name=f"ge{gi}", tag=f"ge{gi}")
                nc.gpsimd.indirect_dma_start(
                    out=ge_tiles[gi][:],
                    out_offset=None,
                    in_=x_d[:],
                    in_offset=bass.IndirectOffsetOnAxis(
                        ap=eq_sb[:, cc : cc + 1], axis=0
                    ),
                    compute_op=(
                        mybir.AluOpType.bypass if t == 0 else mybir.AluOpType.add
                    ),
                )
                if t == rounds_per_group[gi] - 1:
                    nc.vector.tensor_scalar_mul(
                        out=ge_tiles[gi][:],
                        in0=ge_tiles[gi][:],
                        scalar1=ew_sb[:, gi : gi + 1],
                    )
                    mul_done.add(gi)
            assert mul_done == set(range(n_groups))

            if n_groups:
                # all base stores (and extras sums) must land before the
                # scatter-add RMW; dsts are globally unique across groups,
                # so inside the critical section the scatters carry no
                # mutual deps and the SWDGE queue drains them concurrently.
                tc.strict_bb_all_engine_barrier()
                scat_sem = nc.alloc_semaphore("scat_sem")
                with tc.tile_critical():
                    for gi in range(n_groups):
                        nc.gpsimd.indirect_dma_start(
                            out=out_d[:],
                            out_offset=bass.IndirectOffsetOnAxis(
                                ap=ed_sb[:, gi : gi + 1], axis=0
                            ),
                            in_=ge_tiles[gi][:],
                            in_offset=None,
                            compute_op=mybir.AluOpType.add,
                        ).then_inc(scat_sem, 16)
                    nc.gpsimd.wait_ge(scat_sem, 16 * n_groups)

    nc.compile()
    return nc


def _get_built(rounds_per_group: tuple):
    if rounds_per_group not in _BUILD_CACHE:
        _BUILD_CACHE[rounds_per_group] = _build(rounds_per_group)
    return _BUILD_CACHE[rounds_per_group]


def _preprocess_example(p: np.ndarray):
    """p: [T] int32 -> (anchors [T/L], wslot [T], extras dict
    dst_row -> (weight, [src rows]))."""
    change = np.empty(T, dtype=bool)
    change[0] = True
    change[1:] = p[1:] != p[:-1]
    starts = np.flatnonzero(change)
    r = starts.size
    counts = np.diff(np.append(starts, T))
    labels = p[starts]
    w = np.where(labels != BLANK_IDX, 1.0 / counts, 0.0).astype(np.float32)

    g = np.zeros(T, dtype=np.int64)
    c = np.ones(T, dtype=np.int64)
    wv = np.zeros(T, dtype=np.float32)
    g[:r] = starts
    c[:r] = counts
    wv[:r] = w

    n_blocks = T // L
    anchors = np.zeros(n_blocks, dtype=np.int64)
    wslot = np.zeros(T, dtype=np.float32)
    extras = {}

    lvec = np.arange(L)
    votes_all = g.reshape(n_blocks, L) - lvec
    valid_all = wv.reshape(n_blocks, L) > 0

    for bk in range(n_blocks):
        valid = valid_all[bk]
        if not valid.any():
            continue
        votes = votes_all[bk][valid]
        vals, cnts = np.unique(votes, return_counts=True)
        q = int(vals[np.argmax(cnts)])
        q = min(q, T - L)
        anchors[bk] = q
        o0 = bk * L
        for l in range(L):
            o = o0 + l
            if wv[o] <= 0:
                continue
            matched = g[o] == q + l
            if matched:
                wslot[o] = wv[o]
                ks = range(1, int(c[o]))
            else:
                ks = range(0, int(c[o]))
            rows = [int(g[o]) + k for k in ks]
            if rows:
                extras[o] = (float(wv[o]), rows)
    return anchors, wslot, extras


def _make_inputs(x: np.ndarray, preds: np.ndarray):
    """Full inputs -> (in_maps per core, rounds_per_group)."""
    per_ex = [_preprocess_example(np.asarray(preds[b])) for b in range(B)]

    # per core: entries (dst, w, [srcs]) sorted by descending len(srcs)
    core_entries = []
    for cidx in range(N_CORES):
        ents = []
        for e in range(EX_PER_CORE):
            b = EX_PER_CORE * cidx + e
            off = e * T
            for o, (wt, rows) in per_ex[b][2].items():
                ents.append((off + o, wt, [off + s for s in rows]))
        ents.sort(key=lambda t: -len(t[2]))
        core_entries.append(ents)

    # uniform group/round structure across cores (single NEFF)
    max_ents = max((len(e) for e in core_entries), default=0)
    n_groups = (max_ents + 127) // 128
    rounds_per_group = []
    for gi in range(n_groups):
        r = 1
        for ents in core_entries:
            grp = ents[gi * 128 : (gi + 1) * 128]
            if grp:
                r = max(r, max(len(t[2]) for t in grp))
        rounds_per_group.append(int(r))
    rounds_per_group = tuple(rounds_per_group)
    n_round_cols = int(sum(rounds_per_group))

    in_maps = []
    for cidx in range(N_CORES):
        b0 = EX_PER_CORE * cidx
        bidx = np.zeros((128, NTB), dtype=np.int32)
        bw = np.zeros((128, NTB * L), dtype=np.float32)
        for e in range(EX_PER_CORE):
            anchors, wslot, _ = per_ex[b0 + e]
            for bk in range(T // L):
                orow = e * T + bk * L
                j = orow // (128 * L)
                prt = (orow % (128 * L)) // L
                bidx[prt, j] = anchors[bk] + e * T
                bw[prt, j * L : (j + 1) * L] = wslot[bk * L : (bk + 1) * L]

        xc = np.empty((XR, D), dtype=np.float32)
        xc[:ROWS] = np.asarray(x[b0 : b0 + EX_PER_CORE], dtype=np.float32).reshape(
            ROWS, D
        )
        xc[ROWS] = 0.0
        im = {"x": xc, "bidx": bidx, "bw": bw}

        if n_groups:
            eq = np.full((128, n_round_cols), ZROW, dtype=np.int32)
            ew = np.zeros((128, n_groups), dtype=np.float32)
            ed = np.full((128, n_groups), ROWS - 1, dtype=np.int32)
            ents = core_entries[cidx]
            col = 0
            for gi, rounds in enumerate(rounds_per_group):
                grp = ents[gi * 128 : (gi + 1) * 128]
                for i, (dst, wt, srcs) in enumerate(grp):
                    ew[i, gi] = wt
                    ed[i, gi] = dst
                    for t, s in enumerate(srcs):
                        eq[i, col + t] = s
                col += rounds
            im.update({"eq": eq, "ew": ew, "ed": ed})
        in_maps.append(im)
    return in_maps, rounds_per_group


def _run(in_maps, rounds_per_group, trace=False):
    from concourse.bass_utils import run_bass_kernel_spmd

    nc = _get_built(rounds_per_group)
    return run_bass_kernel_spmd(nc, in_maps, list(range(N_CORES)), trace=trace)


def kernel(x: np.ndarray, preds: np.ndarray) -> np.ndarray:
    x = np.asarray(x)
    preds = np.asarray(preds)
    in_maps, rounds_per_group = _make_inputs(x, preds)
    res = _run(in_maps, rounds_per_group)
    out = np.empty((B, T, D), dtype=np.float32)
    for c in range(N_CORES):
        oc = res.results[c]["out"].reshape(EX_PER_CORE, T, D)
        out[EX_PER_CORE * c : EX_PER_CORE * (c + 1)] = oc
    return out


# revision 13
# speedup vs baseline: 1.1972x; 1.1972x over previous
"""Trainium2 Bass kernel for nn_Compressor (consecutive-run mean-pool compressor).

Semantics per batch element (T=4096, D=1024, blank_idx=0):
  - split preds[b] into consecutive runs
  - out[s] = mean(x[start_s : start_s+count_s]) for run s, zeroed when the
    run's label == 0; rows s >= n_runs stay zero (padding to T rows).

Strategy (pure data parallel, 2 examples per core on 8 cores):
  preds is tiny (256 KB) -> run metadata is computed on host with numpy and
  shipped as small index/weight tensors; the 512 MB of x moves on-device.
  Each core's x shard gets one extra all-zero row appended (index ROWS)
  used as a null target for padding gathers.

  Since preds is uniform over 32 labels, ~97% of runs have length 1, so the
  output is nearly a row-gather of x. Per 512 output rows (one block-tile):
    - ONE indirect DMA gathers 128 blocks of L=4 consecutive x rows (16 KB
      per descriptor, keeping SWDGE descriptor generation cheap). Block
      anchors are majority-voted on host so ~97% of output rows are covered
      by the block slot at their position.
    - 4 per-slot DVE multiplies apply w_s = (label!=0)/count_s (0 for
      blank/padding/mismatched rows).
    - ONE blocked 2 MB store (16 KB per descriptor).
  Output rows not fully covered by their slot (anchor mismatch, or runs
  with count>1) get the missing contribution via a compact path: entries
  are grouped by destination row (<=128 unique dsts per group, sorted by
  descending row count); each group's row-sums are built with a plain
  gather plus accumulate-gathers (DMA CCE add into SBUF; exhausted rows
  point at the zero row), scaled once by w, then indirect scatter-added
  (CCE add into DRAM) after the base stores.
"""

import numpy as np

B, T, D = 16, 4096, 1024
N_CORES = 8
EX_PER_CORE = B // N_CORES  # 2
ROWS = EX_PER_CORE * T  # 8192 rows of out per core
XR = ROWS + 1  # x shard rows incl. trailing zero row
ZROW = ROWS  # index of the zero row
L = 4  # rows per gather/store block
NTB = ROWS // (128 * L)  # 16 block-tiles per core
BLANK_IDX = 0

_BUILD_CACHE: dict = {}


def _build(rounds_per_group: tuple):
    """Build + compile the Bass kernel. rounds_per_group[g] = number of
    gather rounds (1 plain + R-1 accumulate) for extras group g."""
    import concourse.bass as bass
    import concourse.tile as tile
    from concourse import bacc, mybir
    # add_dep_helper no longer needed

    n_groups = len(rounds_per_group)
    n_round_cols = int(sum(rounds_per_group))

    nc = bacc.Bacc(
        "TRN2", target_bir_lowering=False, debug=False, enable_asserts=False
    )
    f32, i32 = mybir.dt.float32, mybir.dt.int32

    x_d = nc.dram_tensor("x", [XR, D], f32, kind="ExternalInput").ap()
    bidx_d = nc.dram_tensor("bidx", [128, NTB], i32, kind="ExternalInput").ap()
    bw_d = nc.dram_tensor("bw", [128, NTB * L], f32, kind="ExternalInput").ap()
    out_d = nc.dram_tensor("out", [ROWS, D], f32, kind="ExternalOutput").ap()
    if n_groups:
        eq_d = nc.dram_tensor(
            "eq", [128, n_round_cols], i32, kind="ExternalInput"
        ).ap()
        ew_d = nc.dram_tensor("ew", [128, n_groups], f32, kind="ExternalInput").ap()
        ed_d = nc.dram_tensor("ed", [128, n_groups], i32, kind="ExternalInput").ap()

    with tile.TileContext(nc) as tc:
        with (
            tc.tile_pool(name="const", bufs=1) as cpool,
            tc.tile_pool(name="gather", bufs=6) as gpool,
            tc.tile_pool(name="extras", bufs=max(1, n_groups)) as epool,
        ):
            idx_sb = cpool.tile([128, NTB], i32)
            nc.sync.dma_start(idx_sb[:], bidx_d[:])
            w_sb = cpool.tile([128, NTB * L], f32)
            nc.sync.dma_start(w_sb[:], bw_d[:])
            if n_groups:
                eq_sb = cpool.tile([128, n_round_cols], i32)
                nc.sync.dma_start(eq_sb[:], eq_d[:])
                ew_sb = cpool.tile([128, n_groups], f32)
                nc.sync.dma_start(ew_sb[:], ew_d[:])
                ed_sb = cpool.tile([128, n_groups], i32)
                nc.sync.dma_start(ed_sb[:], ed_d[:])

            # interleave schedule: emit extras gather rounds early in the
            # main loop so the gpsimd engine issues them while store slots
            # are still draining, not after all base gathers.
            round_list = []  # (group, round_t, col)
            col = 0
            for gi in range(n_groups):
                for t in range(rounds_per_group[gi]):
                    round_list.append((gi, t, col))
                    col += 1
            # spread each group's rounds >=n_groups tiles apart: round t of
            # group gi goes after tile 1+gi+t*n_groups, so the serial RMW
            # chain on a group's tile never stalls the in-order gpsimd
            # issue stream (the previous round has long completed).
            max_rounds = max(rounds_per_group) if n_groups else 1
            spread = max(1, min(max(1, n_groups),
                                (NTB - 2 - n_groups) // max(1, max_rounds)))
            rounds_after_tile = {}
            for gi, t, cc in round_list:
                slot = min(1 + gi + t * spread, NTB - 1)
                rounds_after_tile.setdefault(slot, []).append((gi, t, cc))

            ge_tiles = [None] * n_groups
            mul_done = set()

            # main loop: blocked gather -> per-slot scale -> blocked store
            for j in range(NTB):
                g = gpool.tile([128, L * D], f32)
                nc.gpsimd.indirect_dma_start(
                    out=g[:],
                    out_offset=None,
                    in_=x_d[:],
                    in_offset=bass.IndirectOffsetOnAxis(
                        ap=idx_sb[:, j : j + 1], axis=0
                    ),
                )
                for l in range(L):
                    nc.vector.tensor_scalar_mul(
                        out=g[:, l * D : (l + 1) * D],
                        in0=g[:, l * D : (l + 1) * D],
                        scalar1=w_sb[:, j * L + l : j * L + l + 1],
                    )
                nc.sync.dma_start(
                    out_d[j * 128 * L : (j + 1) * 128 * L, :].rearrange(
                        "(p l) d -> p (l d)", l=L
                    ),
                    g[:],
                )
                for gi, t, cc in rounds_after_tile.get(j, []):
                    if ge_tiles[gi] is None:
                        ge_tiles[gi] = epool.tile([128, D], f32, name=f"ge{gi}", tag=f"ge{gi}")
                    nc.gpsimd.indirect_dma_start(
                        out=ge_tiles[gi][:],
                        out_offset=None,
                        in_=x_d[:],
                        in_offset=bass.IndirectOffsetOnAxis(
                            ap=eq_sb[:, cc : cc + 1], axis=0
                        ),
                        compute_op=(
                            mybir.AluOpType.bypass if t == 0 else mybir.AluOpType.add
                        ),
                    )
                    if t == rounds_per_group[gi] - 1:
                        nc.vector.tensor_scalar_mul(
                            out=ge_tiles[gi][:],
                            in0=ge_tiles[gi][:],
                            scalar1=ew_sb[:, gi : gi + 1],
                        )
                        mul_done.add(gi)

            # any rounds not emitted during the loop (slot beyond last tile)
            for gi, t, cc in round_list:
                slot = min(1 + gi + t * spread, NTB - 1)
                if slot <= NTB - 1:
                    continue
                if ge_tiles[gi] is None:
                    ge_tiles[gi] = epool.tile([128, D], f32, name=f"ge{gi}", tag=f"ge{gi}")
                nc.gpsimd.indirect_dma_start(
                    out=ge_tiles[gi][:],
                    out_offset=None,
                    in_=x_d[:],
                    in_offset=bass.IndirectOffsetOnAxis(
                        ap=eq_sb[:, cc : cc + 1], axis=0
                    ),
                    compute_op=(
                        mybir.AluOpType.bypass if t == 0 else mybir.AluOpType.add
                    ),
                )
                if t == rounds_per_group[gi] - 1:
                    nc.vector.tensor_scalar_mul(
                        out=ge_tiles[gi][:],
                        in0=ge_tiles[gi][:],
                        scalar1=ew_sb[:, gi : gi + 1],
                    )
                    mul_done.add(gi)
            assert mul_done == set(range(n_groups))

            if n_groups:
                # all base stores (and extras sums) must land before the
                # scatter-add RMW; dsts are globally unique across groups,
                # so inside the critical section the scatters carry no
                # mutual deps and the SWDGE queue drains them concurrently.
                tc.strict_bb_all_engine_barrier()
                scat_sem = nc.alloc_semaphore("scat_sem")
                with tc.tile_critical():
                    for gi in range(n_groups):
                        nc.gpsimd.indirect_dma_start(
                            out=out_d[:],
                            out_offset=bass.IndirectOffsetOnAxis(
                                ap=ed_sb[:, gi : gi + 1], axis=0
                            ),
                            in_=ge_tiles[gi][:],
                            in_offset=None,
                            compute_op=mybir.AluOpType.add,
                        ).then_inc(scat_sem, 16)
                    nc.gpsimd.wait_ge(scat_sem, 16 * n_groups)

    nc.compile()
    return nc


def _get_built(rounds_per_group: tuple):
    if rounds_per_group not in _BUILD_CACHE:
        _BUILD_CACHE[rounds_per_group] = _build(rounds_per_group)
    return _BUILD_CACHE[rounds_per_group]


def _preprocess_example(p: np.ndarray):
    """p: [T] int32 -> (anchors [T/L], wslot [T], extras dict
    dst_row -> (weight, [src rows]))."""
    change = np.empty(T, dtype=bool)
    change[0] = True
    change[1:] = p[1:] != p[:-1]
    starts = np.flatnonzero(change)
    r = starts.size
    counts = np.diff(np.append(starts, T))
    labels = p[starts]
    w = np.where(labels != BLANK_IDX, 1.0 / counts, 0.0).astype(np.float32)

    g = np.zeros(T, dtype=np.int64)
    c = np.ones(T, dtype=np.int64)
    wv = np.zeros(T, dtype=np.float32)
    g[:r] = starts
    c[:r] = counts
    wv[:r] = w

    n_blocks = T // L
    anchors = np.zeros(n_blocks, dtype=np.int64)
    wslot = np.zeros(T, dtype=np.float32)
    extras = {}

    lvec = np.arange(L)
    votes_all = g.reshape(n_blocks, L) - lvec
    valid_all = wv.reshape(n_blocks, L) > 0

    for bk in range(n_blocks):
        valid = valid_all[bk]
        if not valid.any():
            continue
        votes = votes_all[bk][valid]
        vals, cnts = np.unique(votes, return_counts=True)
        q = int(vals[np.argmax(cnts)])
        q = min(q, T - L)
        anchors[bk] = q
        o0 = bk * L
        for l in range(L):
            o = o0 + l
            if wv[o] <= 0:
                continue
            matched = g[o] == q + l
            if matched:
                wslot[o] = wv[o]
                ks = range(1, int(c[o]))
            else:
                ks = range(0, int(c[o]))
            rows = [int(g[o]) + k for k in ks]
            if rows:
                extras[o] = (float(wv[o]), rows)
    return anchors, wslot, extras


def _make_inputs(x: np.ndarray, preds: np.ndarray):
    """Full inputs -> (in_maps per core, rounds_per_group)."""
    per_ex = [_preprocess_example(np.asarray(preds[b])) for b in range(B)]

    # per core: entries (dst, w, [srcs]) sorted by descending len(srcs)
    core_entries = []
    for cidx in range(N_CORES):
        ents = []
        for e in range(EX_PER_CORE):
            b = EX_PER_CORE * cidx + e
            off = e * T
            for o, (wt, rows) in per_ex[b][2].items():
                ents.append((off + o, wt, [off + s for s in rows]))
        ents.sort(key=lambda t: -len(t[2]))
        core_entries.append(ents)

    # uniform group/round structure across cores (single NEFF)
    max_ents = max((len(e) for e in core_entries), default=0)
    n_groups = (max_ents + 127) // 128
    rounds_per_group = []
    for gi in range(n_groups):
        r = 1
        for ents in core_entries:
            grp = ents[gi * 128 : (gi + 1) * 128]
            if grp:
                r = max(r, max(len(t[2]) for t in grp))
        rounds_per_group.append(int(r))
    rounds_per_group = tuple(rounds_per_group)
    n_round_cols = int(sum(rounds_per_group))

    in_maps = []
    for cidx in range(N_CORES):
        b0 = EX_PER_CORE * cidx
        bidx = np.zeros((128, NTB), dtype=np.int32)
        bw = np.zeros((128, NTB * L), dtype=np.float32)
        for e in range(EX_PER_CORE):
            anchors, wslot, _ = per_ex[b0 + e]
            for bk in range(T // L):
                orow = e * T + bk * L
                j = orow // (128 * L)
                prt = (orow % (128 * L)) // L
                bidx[prt, j] = anchors[bk] + e * T
                bw[prt, j * L : (j + 1) * L] = wslot[bk * L : (bk + 1) * L]

        xc = np.empty((XR, D), dtype=np.float32)
        xc[:ROWS] = np.asarray(x[b0 : b0 + EX_PER_CORE], dtype=np.float32).reshape(
            ROWS, D
        )
        xc[ROWS] = 0.0
        im = {"x": xc, "bidx": bidx, "bw": bw}

        if n_groups:
            eq = np.full((128, n_round_cols), ZROW, dtype=np.int32)
            ew = np.zeros((128, n_groups), dtype=np.float32)
            ed = np.full((128, n_groups), ROWS - 1, dtype=np.int32)
            ents = core_entries[cidx]
            col = 0
            for gi, rounds in enumerate(rounds_per_group):
                grp = ents[gi * 128 : (gi + 1) * 128]
                for i, (dst, wt, srcs) in enumerate(grp):
                    ew[i, gi] = wt
                    ed[i, gi] = dst
                    for t, s in enumerate(srcs):
                        eq[i, col + t] = s
                col += rounds
            im.update({"eq": eq, "ew": ew, "ed": ed})
        in_maps.append(im)
    return in_maps, rounds_per_group


def _run(in_maps, rounds_per_group, trace=False):
    from concourse.bass_utils import run_bass_kernel_spmd

    nc = _get_built(rounds_per_group)
    return run_bass_kernel_spmd(nc, in_maps, list(range(N_CORES)), trace=trace)


def kernel(x: np.ndarray, preds: np.ndarray) -> np.ndarray:
    x = np.asarray(x)
    preds = np.asarray(preds)
    in_maps, rounds_per_group = _make_inputs(x, preds)
    res = _run(in_maps, rounds_per_group)
    out = np.empty((B, T, D), dtype=np.float32)
    for c in range(N_CORES):
        oc = res.results[c]["out"].reshape(EX_PER_CORE, T, D)
        out[EX_PER_CORE * c : EX_PER_CORE * (c + 1)] = oc
    return out
